# revision 1
# baseline (speedup 1.0000x reference)
"""BitNet-style attention layer (B=2, T=2048, D=1024, 16 heads, RoPE, causal)
on 8 TRN2 NeuronCores.

Sharding: head-parallel attention (2 heads/core); wo is computed per-core for
an o-slice after an AllGather of the int8-valued (bf16-stored) quantized
attention output.  One small AllReduce(max) provides the wo-input
quantization scale.
"""

import math
from contextlib import ExitStack

import ml_dtypes
import numpy as np

import concourse.bass as bass
import concourse.bacc as bacc_mod
import concourse.mybir as mybir
import concourse.tile as tile
from concourse.bass_utils import run_bass_kernel_spmd

F32 = mybir.dt.float32
F32R = mybir.dt.float32r
BF16 = mybir.dt.bfloat16
OP = mybir.AluOpType

B, T, D = 2, 2048, 1024
NT = B * T              # 4096 tokens
NH, HD = 16, 64
N_CORES = 8
HPC = NH // N_CORES     # heads per core = 2
DPC = HPC * HD          # dims per core = 128
RC = 12582912.0         # 1.5*2^23: round-to-nearest-even constant
NEG = -1e30

TB = 512                # token block (matmul N)
NTB = NT // TB          # 8
NTT = NT // 128         # 32 token tiles
QB = 512                # q block
NQB = T // QB           # 4 per batch
NKT = T // 128          # 16 k tiles per batch


def _quant_w(w):
    O, I = w.shape
    wg = w.reshape(O, I // 128, 128)
    ws = np.abs(wg).mean(-1, keepdims=True) + 1e-5
    wq = np.clip(np.round(wg / ws), -1.0, 1.0) * ws
    return wq.reshape(O, I).astype(np.float32)


def build_nc():
    nc = bacc_mod.Bacc(num_devices=N_CORES)
    io = {}

    def inp(name, shape, dt=F32):
        io[name] = nc.dram_tensor(name, shape, dt, kind="ExternalInput")

    inp("x", [NT, D])
    inp("sxp", [128, NTT])
    inp("isx", [128, NT])
    inp("wqT", [D, DPC], F32R)
    inp("wkT", [D, DPC], F32R)
    inp("wvT", [D, DPC], F32R)
    inp("woT", [D, DPC], BF16)
    inp("cmap", [128, NT])
    inp("smap", [128, NT])
    inp("pswapT", [128, 128], F32R)
    inp("negI", [128, 128], BF16)
    inp("umask", [128, 4 * QB], BF16)
    inp("sel2", [33, 128], F32R)
    inp("ones1", [1, 128], F32R)
    inp("ident", [128, 128], F32R)
    out = nc.dram_tensor("out", [DPC, NT], F32, kind="ExternalOutput")

    r32 = lambda ap: ap.bitcast(F32R)
    RG = [list(range(N_CORES))]

    with nc.allow_low_precision(reason="f32r matmul pipeline (FP22 mantissa is ample here)"), \
         tile.TileContext(nc) as tc, ExitStack() as top:
        cpool = top.enter_context(tc.tile_pool(name="const", bufs=1))
        dpool = top.enter_context(tc.tile_pool(name="dram", bufs=1, space="DRAM"))

        # ---- constants
        def const_tile(name, shape, dt=F32, src=None):
            t = cpool.tile(shape, dt, tag=name)
            nc.gpsimd.dma_start(t[:], src if src is not None else io[name][:])
            return t

        wq_sb = [const_tile(f"wq{i}", [128, DPC], F32R, io["wqT"][i * 128:(i + 1) * 128, :]) for i in range(8)]
        wk_sb = [const_tile(f"wk{i}", [128, DPC], F32R, io["wkT"][i * 128:(i + 1) * 128, :]) for i in range(8)]
        wv_sb = [const_tile(f"wv{i}", [128, DPC], F32R, io["wvT"][i * 128:(i + 1) * 128, :]) for i in range(8)]
        sxp = const_tile("sxp", [128, NTT])
        pswapT = const_tile("pswapT", [128, 128], F32R)
        negI = const_tile("negI", [128, 128], BF16)
        umask = const_tile("umask", [128, 4 * QB], BF16)
        sel2 = const_tile("sel2", [33, 128], F32R)
        ones1 = const_tile("ones1", [1, 128], F32R)
        ident = const_tile("ident", [128, 128], F32R)

        # ---- lifetime-scoped big buffers (opened in reverse-close order)
        es_big1 = ExitStack()
        big1 = es_big1.enter_context(tc.tile_pool(name="big1", bufs=1))
        es_va = ExitStack()
        vap = es_va.enter_context(tc.tile_pool(name="vap", bufs=1))
        es_qkv = ExitStack()
        qkvp = es_qkv.enter_context(tc.tile_pool(name="qkv", bufs=1))
        qT = qkvp.tile([128, NT], F32R, name="qT", tag="qT")
        kT = qkvp.tile([128, NT], F32R, name="kT", tag="kT")
        vT = qkvp.tile([128, NT], F32R, name="vT", tag="vT")
        xiT = [big1.tile([128, NT], BF16, name=f"xiT{i}", tag=f"xiT{i}") for i in range(8)]

        # ======== P1: x quantization (natural layout) + transpose DMAs
        xi2d = dpool.tile([NT, D], BF16, name="xi2d", tag="xi2d")
        with tc.tile_pool(name="p1", bufs=2) as p1:
            for tt in range(NTT):
                xt = p1.tile([128, D], F32, name="xt", tag="xt")
                nc.gpsimd.dma_start(xt[:], io["x"][tt * 128:(tt + 1) * 128, :])
                y = p1.tile([128, D], F32, name="y", tag="y")
                nc.gpsimd.tensor_scalar(y[:], xt[:], sxp[:, tt:tt + 1], RC,
                                        OP.mult, OP.add)
                xi = p1.tile([128, D], BF16, name="xi", tag="xi")
                nc.gpsimd.tensor_scalar(xi[:], y[:], RC, None, OP.subtract)
                nc.gpsimd.dma_start(xi2d[tt * 128:(tt + 1) * 128, :], xi[:])
                for i in range(8):
                    nc.sync.dma_start(xiT[i][:, tt * 128:(tt + 1) * 128],
                                      xi2d[tt * 128:(tt + 1) * 128,
                                           i * 128:(i + 1) * 128],
                                      transpose=True)

        # ======== P3: projections q,k,v (f32r); fold 1/s_x into copy-out
        with tc.tile_pool(name="p3", bufs=2) as p3, \
             tc.tile_pool(name="p3p", bufs=3, space="PSUM") as p3p, \
             tc.tile_pool(name="p3i", bufs=2) as p3i:
            for tb in range(NTB):
                sl = slice(tb * TB, (tb + 1) * TB)
                isxb = p3i.tile([128, TB], F32, name="isxb", tag="isxb")
                nc.sync.dma_start(isxb[:], io["isx"][:, sl])
                xf = []
                for i in range(8):
                    t = p3.tile([128, TB], F32R, name=f"xf{i}", tag=f"xf{i}")
                    nc.vector.tensor_copy(t[:], xiT[i][:, sl])
                    xf.append(t)
                for w_sb, dstT in ((wq_sb, qT), (wk_sb, kT), (wv_sb, vT)):
                    pp = p3p.tile([128, TB], F32, name="pp", tag="pp")
                    for i in range(8):
                        nc.tensor.matmul(pp[:], w_sb[i][:], xf[i][:],
                                         start=(i == 0), stop=(i == 7))
                    nc.vector.tensor_tensor(dstT[:, sl], pp[:], isxb[:], OP.mult)

        # ======== P4: RoPE on q, k
        qR = big1.tile([128, NT], F32R, name="qR", tag="xiT0")
        kR = big1.tile([128, NT], F32R, name="kR", tag="xiT1")
        with tc.tile_pool(name="p4", bufs=3) as p4, \
             tc.tile_pool(name="p4p", bufs=3, space="PSUM") as p4p, \
             tc.tile_pool(name="p4m", bufs=3) as p4m:
            for tb in range(NTB):
                sl = slice(tb * TB, (tb + 1) * TB)
                cm = p4m.tile([128, TB], F32, name="cm", tag="cm")
                nc.sync.dma_start(cm[:], io["cmap"][:, sl])
                sm = p4m.tile([128, TB], F32, name="sm", tag="sm")
                nc.sync.dma_start(sm[:], io["smap"][:, sl])
                for srcT, dstR in ((qT, qR), (kT, kR)):
                    swp = p4p.tile([128, TB], F32, name="swp", tag="swp")
                    nc.tensor.matmul(swp[:], pswapT[:], srcT[:, sl],
                                     start=True, stop=True)
                    tmp = p4.tile([128, TB], F32, name="tmp", tag="tmp")
                    nc.vector.tensor_tensor(tmp[:], srcT[:, sl], cm[:], OP.mult)
                    tmp2 = p4.tile([128, TB], F32, name="tmp2", tag="tmp2")
                    nc.vector.tensor_tensor(tmp2[:], swp[:], sm[:], OP.mult)
                    nc.gpsimd.tensor_tensor(dstR[:, sl], tmp[:], tmp2[:], OP.add)

        # ======== P5: V transpose to natural layout (bf16)
        ones_col = cpool.tile([128, 1], BF16, name="ones_col", tag="ones_col")
        nc.vector.memset(ones_col[:], 1.0)
        vaug = [[None] * NTT for _ in range(HPC)]
        with tc.tile_pool(name="p5p", bufs=3, space="PSUM") as p5p:
            for kt in range(NTT):
                vtp = p5p.tile([128, 128], F32, name="vtp", tag="vtp")
                nc.tensor.transpose(r32(vtp[:]), vT[:, kt * 128:(kt + 1) * 128],
                                    ident[:])
                for h in range(HPC):
                    va = vap.tile([128, HD], BF16, name=f"va{h}_{kt}", tag=f"va{h}_{kt}")
                    if h == 0:
                        nc.scalar.copy(va[:], vtp[:, 0:HD])
                    else:
                        nc.vector.tensor_copy(va[:], vtp[:, HD:128])
                    vaug[h][kt] = va
        es_qkv.close()

        # ======== P6: attention
        out_n = big1.tile([128, NT], F32R, name="out_n", tag="xiT2")
        with tc.tile_pool(name="p6a", bufs=3) as p6a, \
             tc.tile_pool(name="p6s", bufs=2, space="PSUM") as p6s, \
             tc.tile_pool(name="p6o", bufs=1, space="PSUM") as p6o:
            for b in range(B):
                for qb in range(NQB):
                    qsl = slice(b * T + qb * QB, b * T + (qb + 1) * QB)
                    nkt = 4 * qb + 4
                    psA0 = p6o.tile([128, QB], F32, name="psA0", tag="psA0")
                    psA1 = p6o.tile([128, QB], F32, name="psA1", tag="psA1")
                    psRS0 = p6o.tile([1, QB], F32, name="psRS0", tag="psRS0")
                    psRS1 = p6o.tile([33, QB], F32, name="psRS1", tag="psRS1")
                    for kl in range(nkt):
                        kt = b * NKT + kl
                        ksl = slice(kt * 128, (kt + 1) * 128)
                        psS = p6s.tile([128, 2 * QB], F32, name="psS", tag="psS")
                        diag = kl >= 4 * qb
                        for h in range(HPC):
                            hsl = slice(h * HD, (h + 1) * HD)
                            ssl = slice(h * QB, (h + 1) * QB)
                            nc.tensor.matmul(psS[:, ssl], kR[hsl, ksl],
                                             qR[hsl, qsl],
                                             start=True, stop=not diag)
                            if diag:
                                v = kl - 4 * qb
                                nc.tensor.matmul(
                                    psS[:, ssl], negI[:],
                                    umask[:, v * QB:(v + 1) * QB],
                                    start=False, stop=True)
                        A = p6a.tile([128, 2 * QB], BF16, name="A", tag="A")
                        nc.scalar.activation(A[:], psS[:],
                                             mybir.ActivationFunctionType.Exp,
                                             scale=1.0 / math.sqrt(HD))
                        st, sp = kl == 0, kl == nkt - 1
                        nc.tensor.matmul(psA0[0:HD, :], vaug[0][kt][:],
                                         A[:, 0:QB], start=st, stop=sp,
                                         tile_position=(0, 0))
                        nc.tensor.matmul(psA1[HD:128, :], vaug[1][kt][:],
                                         A[:, QB:2 * QB], start=st, stop=sp,
                                         tile_position=(0, 64))
                        nc.tensor.matmul(psRS0[0:1, :], ones_col[:],
                                         A[:, 0:QB], start=st, stop=sp,
                                         tile_position=(0, 0))
                        nc.tensor.matmul(psRS1[32:33, :], ones_col[:],
                                         A[:, QB:2 * QB], start=st, stop=sp,
                                         tile_position=(0, 32))
                    ou_blk = p6a.tile([128, QB], F32, name="ou_blk", tag="ou_blk")
                    nc.scalar.copy(ou_blk[0:HD, :], psA0[0:HD, :])
                    nc.scalar.copy(ou_blk[HD:128, :], psA1[HD:128, :])
                    rsi = p6a.tile([33, QB], F32R, name="rsi", tag="rsi")
                    nc.vector.tensor_copy(rsi[0:32, :], umask[0:32, 0:QB])
                    nc.vector.reciprocal(rsi[0:1, :], psRS0[0:1, :])
                    nc.vector.reciprocal(rsi[32:33, :], psRS1[32:33, :])
                    brs = p6s.tile([128, QB], F32, name="brs", tag="psS")
                    nc.tensor.matmul(brs[:], sel2[:], rsi[:],
                                     start=True, stop=True)
                    nc.vector.tensor_tensor(out_n[:, qsl], ou_blk[:], brs[:],
                                            OP.mult)
        es_va.close()

        # ======== P7: out-quant + collectives + wo
        xio = big1.tile([128, NT], BF16, name="xio", tag="xiT3")
        with tc.tile_pool(name="p7p", bufs=2, space="PSUM") as p7p, \
             tc.tile_pool(name="p7", bufs=1) as p7:
            # out-quant absmax over partition dim via PE transpose
            amax = p7.tile([128, NTT], F32, name="amax", tag="amax")
            for ot in range(NTT):
                tp = p7p.tile([128, 128], F32, name="tp", tag="tp")
                nc.tensor.transpose(r32(tp[:]), out_n[:, ot * 128:(ot + 1) * 128],
                                    ident[:])
                nc.vector.tensor_reduce(amax[:, ot:ot + 1], tp[:],
                                        mybir.AxisListType.X, OP.max,
                                        apply_absolute_value=True)
            ar_in = dpool.tile([128, NTT], F32, name="ar_in", tag="ar_in")
            ar_out = dpool.tile([128, NTT], F32, name="ar_out", tag="ar_out", addr_space="Shared")
            nc.sync.dma_start(ar_in[:], amax[:])
            nc.gpsimd.collective_compute(
                "AllReduce", OP.max, replica_groups=RG,
                ins=[ar_in[:].opt()], outs=[ar_out[:].opt()])
            gmax = p7.tile([128, NTT], F32, name="gmax", tag="gmax")
            nc.sync.dma_start(gmax[:], ar_out[:])
            iso_p = p7.tile([128, NTT], F32, name="iso_p", tag="iso_p")
            nc.vector.tensor_scalar(iso_p[:], gmax[:], 1e-5, 1.0 / 127.0,
                                    OP.add, OP.mult)
            so_p = p7.tile([128, NTT], F32R, name="so_p", tag="so_p")
            nc.vector.reciprocal(so_p[:], iso_p[:])
            sop_t = p7p.tile([NTT, 128], F32, name="tp", tag="tp")
            nc.tensor.transpose(r32(sop_t[:]), so_p[:], ident[:])
            so_sq = p7.tile([NTT, 128], F32R, name="so_sq", tag="so_sq")
            nc.scalar.copy(so_sq[:], sop_t[:])
            so_row = p7.tile([1, NT], F32R, name="so_row", tag="so_row")
            for j in range(NTT):
                nc.sync.dma_start(so_row[0:1, j * 128:(j + 1) * 128],
                                  so_sq[j:j + 1, :])
            iso_row = p7.tile([1, NT], F32R, name="iso_row", tag="iso_row")
            nc.vector.reciprocal(iso_row[:], so_row[:])

            # quantize out_n -> xio (integer-valued bf16)
            for tb in range(NTB):
                sl = slice(tb * TB, (tb + 1) * TB)
                bso = p7p.tile([128, TB], F32, name="brs", tag="brs")
                nc.tensor.matmul(bso[:], ones1[:], so_row[:, sl],
                                 start=True, stop=True)
                yq = p7.tile([128, TB], F32, name=f"yq{tb % 2}", tag=f"yq{tb % 2}")
                nc.vector.tensor_tensor(yq[:], out_n[:, sl], bso[:], OP.mult)
                nc.gpsimd.tensor_scalar(xio[:, sl], yq[:], RC, RC,
                                        OP.add, OP.subtract)

            # ======== P9: AllGather
            ag_in = dpool.tile([128, NT], BF16, name="ag_in", tag="ag_in")
            ag_out = dpool.tile([N_CORES * 128, NT], BF16, name="ag_out", tag="ag_out", addr_space="Shared")
            nc.sync.dma_start(ag_in[:], xio[:])
            nc.gpsimd.collective_compute(
                "AllGather", OP.bypass, replica_groups=RG,
                ins=[ag_in[:].opt()], outs=[ag_out[:].opt()])

            # ======== P10: wo projection (bf16) + final scale
            with tc.tile_pool(name="pA", bufs=2) as pA, \
                 tc.tile_pool(name="pAg", bufs=1) as pAg, \
                 tc.tile_pool(name="pAp", bufs=2, space="PSUM") as pAp:
                wo_sb = [pAg.tile([128, DPC], BF16, name=f"wo{i}", tag=f"wo{i}")
                         for i in range(8)]
                g_sb = [big1.tile([128, NT], BF16, name=f"g{i}", tag=f"xiT{i}") for i in range(8)]
                for i in range(8):
                    nc.sync.dma_start(wo_sb[i][:],
                                      io["woT"][i * 128:(i + 1) * 128, :])
                    nc.sync.dma_start(g_sb[i][:],
                                      ag_out[i * 128:(i + 1) * 128, :])
                iso_bc = pAg.tile([128, NT], F32, name="iso_bc", tag="iso_bc")
                for tb in range(NTB):
                    sl = slice(tb * TB, (tb + 1) * TB)
                    bi = pAp.tile([128, TB], F32, name="bi", tag="bi")
                    nc.tensor.matmul(bi[:], ones1[:], iso_row[:, sl],
                                     start=True, stop=True)
                    nc.scalar.copy(iso_bc[:, sl], bi[:])
                for tb in range(NTB):
                    sl = slice(tb * TB, (tb + 1) * TB)
                    pw = pAp.tile([128, TB], F32, name="pw", tag="pw")
                    for i in range(8):
                        nc.tensor.matmul(pw[:], wo_sb[i][:], g_sb[i][:, sl],
                                         start=(i == 0), stop=(i == 7))
                    fin = pA.tile([128, TB], F32, name="fin", tag="fin")
                    nc.vector.tensor_tensor(fin[:], pw[:], iso_bc[:, sl],
                                            OP.mult)
                    nc.sync.dma_start(out[:, sl], fin[:])
        es_big1.close()

    return nc


_CACHE = {}


def kernel(x, cos, sin, wq_w, wk_w, wv_w, wo_w):
    x = np.asarray(x, np.float32)
    cos = np.asarray(cos, np.float32)   # [T, 32]
    sin = np.asarray(sin, np.float32)
    xf = np.ascontiguousarray(x.reshape(NT, D))

    amax = np.abs(xf).max(-1) + 1e-5
    sx = (127.0 / amax).astype(np.float32)
    isx = (amax / 127.0).astype(np.float32)
    sxp = np.ascontiguousarray(sx.reshape(NTT, 128).T)
    isx_bc = np.ascontiguousarray(np.broadcast_to(isx[None, :], (128, NT)))

    # RoPE maps from the provided cos/sin tables
    cm64 = np.repeat(cos.T, 2, axis=0)            # [64, T]
    sm64 = np.repeat(sin.T, 2, axis=0)
    # rows: [64 dims for head-even][64 dims for head-odd]; cols: [b0 | b1]
    cmap = np.tile(np.concatenate([cm64, cm64], axis=0), (1, B)).astype(np.float32)
    smap = np.tile(np.concatenate([sm64, sm64], axis=0), (1, B)).astype(np.float32)

    P = np.zeros((128, 128), np.float32)
    for j in range(64):
        P[2 * j, 2 * j + 1] = -1.0
        P[2 * j + 1, 2 * j] = 1.0
    pswapT = np.ascontiguousarray(P.T)
    negI = (NEG * np.eye(128)).astype(ml_dtypes.bfloat16)
    kk = np.arange(128)[:, None]
    qq = np.arange(QB)[None, :]
    um = np.concatenate([((v * 128 + kk) > qq).astype(np.float32)
                         for v in range(4)], axis=1).astype(ml_dtypes.bfloat16)
    sel2 = np.zeros((33, 128), np.float32)
    sel2[0, 0:HD] = 1.0
    sel2[32, HD:128] = 1.0
    ones1 = np.ones((1, 128), np.float32)
    ident = np.eye(128, dtype=np.float32)

    wq_e, wk_e, wv_e, wo_e = (_quant_w(np.asarray(w, np.float32))
                              for w in (wq_w, wk_w, wv_w, wo_w))

    if "nc" not in _CACHE:
        nc0 = build_nc()
        nc0.finalize()
        _CACHE["nc"] = nc0
    nc = _CACHE["nc"]

    in_maps = []
    for c in range(N_CORES):
        hs = slice(c * DPC, (c + 1) * DPC)
        in_maps.append({
            "x": xf, "sxp": sxp, "isx": isx_bc,
            "wqT": np.ascontiguousarray(wq_e[hs, :].T),
            "wkT": np.ascontiguousarray(wk_e[hs, :].T),
            "wvT": np.ascontiguousarray(wv_e[hs, :].T),
            "woT": np.ascontiguousarray(wo_e[hs, :].T).astype(ml_dtypes.bfloat16),
            "cmap": cmap, "smap": smap, "pswapT": pswapT, "negI": negI,
            "umask": um, "sel2": sel2, "ones1": ones1, "ident": ident,
        })

    res = run_bass_kernel_spmd(nc, in_maps, core_ids=list(range(N_CORES)))
    outp = np.empty((NT, D), np.float32)
    for c in range(N_CORES):
        outp[:, c * DPC:(c + 1) * DPC] = res.results[c]["out"].T
    return outp.reshape(B, T, D)



# revision 15
# speedup vs baseline: 3.8509x; 3.8509x over previous
"""BitNet-style attention (B=2, T=2048, D=1024, 16 heads, RoPE, causal) on
8 TRN2 NeuronCores.

Head-parallel sharding: 2 heads/core.  Full-bf16 matmul pipeline:
  - host pre-quantizes x (int-valued bf16, transposed) and the ternary
    weights; per-token dequant scales (isx) are folded into the RoPE
    tables (q,k), the exp bias (A' = A*isx_k), and an augmented V column
    (1/isx_k = sx) that yields the softmax denominator for free.
  - scores: 2 heads row-packed on the PE (K=64 each); causal blocks get
    a restricted moving dim; the 128-wide diagonal is masked post-exp.
  - output quant: partition-reduce absmax on GpSimd, AllReduce(max) per
    batch, AllGather of int-valued bf16, per-core wo slice.
"""

import math
from contextlib import ExitStack

import ml_dtypes
import numpy as np

import concourse.bass as bass
import concourse.bacc as bacc_mod
import concourse.bass_isa as bass_isa
import concourse.mybir as mybir
import concourse.tile as tile
from concourse.bass_utils import run_bass_kernel_spmd

F32 = mybir.dt.float32
F32R = mybir.dt.float32r
BF16 = mybir.dt.bfloat16
OP = mybir.AluOpType
ACT = mybir.ActivationFunctionType

B, T, D = 2, 2048, 1024
NT = B * T              # 4096 tokens
NH, HD = 16, 64
N_CORES = 8
HPC = NH // N_CORES     # heads/core = 2
DPC = HPC * HD          # dims/core = 128
RC = 12582912.0         # 1.5*2^23 round-to-nearest-even constant

TB = 512                # token block (matmul N)
NTB = NT // TB          # 8
NTT = NT // 128         # 32 token tiles
QB = 512                # q block
NQB = T // QB           # 4 per batch
NKT = T // 128          # 16 k tiles per batch
VW = 130                # vaug group width: [v_h0(64) | sx | v_h1(64) | sx]
CH = 2048               # collective chunk = one batch (tokens)
NCH = NT // CH          # 2 chunks
ISQ = 1.0 / math.sqrt(HD)


def _quant_w(w):
    O, I = w.shape
    wg = w.reshape(O, I // 128, 128)
    ws = np.abs(wg).mean(-1, keepdims=True) + 1e-5
    wq = np.clip(np.round(wg / ws), -1.0, 1.0) * ws
    return wq.reshape(O, I).astype(np.float32)


def build_nc():
    nc = bacc_mod.Bacc(num_devices=N_CORES)
    io = {}

    def inp(name, shape, dt=F32):
        io[name] = nc.dram_tensor(name, shape, dt, kind="ExternalInput")

    inp("xiT", [D, NT], BF16)        # quantized x, transposed (int-valued)
    inp("wall", [D, 4 * DPC], BF16)  # [wq|wk|wv|wo] transposed slices
    inp("cmx", [128, NT], BF16)      # cos table * isx
    inp("smx", [128, NT], BF16)      # sin table * isx
    inp("lnisx", [128, NTT], F32)    # ln(isx) laid out [token%128, tile]
    inp("sxp", [128, NTT], BF16)     # sx laid out [token%128, tile]
    inp("umask", [128, 128], BF16)   # tri mask (1 if q>=k)
    inp("pswapT", [128, 128], BF16)  # RoPE pair swap
    inp("identb", [128, 128], BF16)
    inp("sel16", [16, 16 * 64], F32R)   # one-hot row selectors (renorm bcast)
    inp("sel4", [4, 4 * 128], F32R)     # one-hot row selectors (scale bcast)
    out = nc.dram_tensor("out", [DPC, NT], F32, kind="ExternalOutput")

    r32 = lambda ap: ap.bitcast(F32R)
    RG = [list(range(N_CORES))]

    with nc.allow_low_precision(reason="bf16 matmul pipeline on int-exact activations"), \
         tile.TileContext(nc) as tc, ExitStack() as top:
        cpool = top.enter_context(tc.tile_pool(name="const", bufs=1))
        dpool = top.enter_context(tc.tile_pool(name="dram", bufs=1, space="DRAM"))

        # ---------------- persistent tiles ----------------
        w_sb = [cpool.tile([128, 4 * DPC], BF16, name=f"w{i}", tag=f"w{i}")
                for i in range(8)]
        xi_sb = [cpool.tile([128, NT], BF16, name=f"xi{i}", tag=f"xi{i}")
                 for i in range(8)]
        lnisx = cpool.tile([128, NTT], F32, name="lnisx", tag="lnisx")
        sxp = cpool.tile([128, NTT], BF16, name="sxp", tag="sxp")
        umask = cpool.tile([128, 128], BF16, name="umask", tag="umask")
        pswapT = cpool.tile([128, 128], BF16, name="pswapT", tag="pswapT")
        identb = cpool.tile([128, 128], BF16, name="identb", tag="identb")
        sel16 = cpool.tile([16, 16 * 64], F32R, name="sel16", tag="sel16")
        sel4 = cpool.tile([4, 4 * 128], F32R, name="sel4", tag="sel4")

        qR = cpool.tile([128, NT], BF16, name="qR", tag="qR")
        kR = cpool.tile([128, NT], BF16, name="kR", tag="kR")
        vaug = cpool.tile([128, NTT * VW], BF16, name="vaug", tag="vaug")
        # unnormalized attention out + denominator row (row 64);
        # head0 cols [0:NT), head1 cols [NT:2NT)
        outU = cpool.tile([65, 2 * NT], F32, name="outU", tag="outU")
        rinv = cpool.tile([16, QB], F32, name="rinv", tag="rinv")
        scl = cpool.tile([4, CH], F32, name="scl", tag="scl")  # so c0,c1; iso c0,c1

        for i in range(8):
            nc.sync.dma_start(w_sb[i][:], io["wall"][i * 128:(i + 1) * 128, :])
        for nm, t in (("lnisx", lnisx), ("sxp", sxp), ("umask", umask),
                      ("pswapT", pswapT), ("identb", identb),
                      ("sel16", sel16), ("sel4", sel4)):
            nc.sync.dma_start(t[:], io[nm][:])
        # staged rows are contracted against one-hot selectors before every
        # row is written; zero-init so 0*garbage can't produce NaN
        nc.vector.memset(rinv[:], 0.0)
        nc.vector.memset(scl[:], 0.0)
        # x loads ordered by token block so projections can start early
        for tb in range(NTB):
            sl = slice(tb * TB, (tb + 1) * TB)
            for i in range(8):
                nc.sync.dma_start(xi_sb[i][:, sl],
                                  io["xiT"][i * 128:(i + 1) * 128, sl])

        # prefill vaug sx columns (cols 64 and 129 of each 130-wide group)
        for kt in range(NTT):
            nc.vector.tensor_copy(vaug[:, kt * VW + 64:kt * VW + 65],
                                  sxp[:, kt:kt + 1])
            nc.vector.tensor_copy(vaug[:, kt * VW + 129:kt * VW + 130],
                                  sxp[:, kt:kt + 1])

        # DRAM scratch for collectives
        ar_in = dpool.tile([1, NT], F32, name="ar_in", tag="ar_in")
        ar_out = [dpool.tile([1, CH], F32, name=f"ar_out{c}", tag=f"ar_out{c}",
                             addr_space="Shared") for c in range(NCH)]
        ag_in = [dpool.tile([128, CH], BF16, name=f"ag_in{c}", tag=f"ag_in{c}")
                 for c in range(NCH)]
        ag_out = [dpool.tile([N_CORES * 128, CH], BF16, name=f"ag_out{c}",
                             tag=f"ag_out{c}", addr_space="Shared")
                  for c in range(NCH)]

        # ================= Phase A: qkv + RoPE + V transpose =================
        with tc.tile_pool(name="pa", bufs=3) as pa, \
             tc.tile_pool(name="pap", bufs=2, space="PSUM") as pap, \
             tc.tile_pool(name="pasw", bufs=2, space="PSUM") as pasw, \
             tc.tile_pool(name="pavt", bufs=2, space="PSUM") as pavt:
            for tb in range(NTB):
                sl = slice(tb * TB, (tb + 1) * TB)
                cmb = pa.tile([128, TB], BF16, name="cmb", tag="cmb")
                nc.sync.dma_start(cmb[:], io["cmx"][:, sl])
                smb = pa.tile([128, TB], BF16, name="smb", tag="smb")
                nc.sync.dma_start(smb[:], io["smx"][:, sl])
                blk = {}
                for pi, pname in enumerate(("q", "k", "v")):
                    pp = pap.tile([128, TB], F32, name="pp", tag="pp")
                    for i in range(8):
                        nc.tensor.matmul(pp[:], w_sb[i][:, pi * 128:(pi + 1) * 128],
                                         xi_sb[i][:, sl],
                                         start=(i == 0), stop=(i == 7))
                    t = pa.tile([128, TB], BF16, name=f"t_{pname}", tag=f"t_{pname}")
                    if pname == "v":
                        nc.vector.tensor_copy(t[:], pp[:])
                    else:
                        nc.scalar.copy(t[:], pp[:])
                    blk[pname] = t
                # --- RoPE on q, k
                for pname, dstR in (("q", qR), ("k", kR)):
                    src = blk[pname]
                    swp = pasw.tile([128, TB], F32, name="swp", tag="swp")
                    nc.tensor.matmul(swp[:], pswapT[:], src[:],
                                     start=True, stop=True)
                    t1 = pa.tile([128, TB], BF16, name="t1", tag="t1")
                    nc.vector.tensor_tensor(t1[:], src[:], cmb[:], OP.mult)
                    t2 = pa.tile([128, TB], BF16, name="t2", tag="t2")
                    nc.vector.tensor_tensor(t2[:], swp[:], smb[:], OP.mult)
                    nc.vector.tensor_tensor(dstR[:, sl], t1[:], t2[:], OP.add)
                # --- V transpose into vaug
                for j in range(TB // 128):
                    kt = tb * 4 + j
                    vtp = pavt.tile([128, 128], BF16, name="vtp", tag="vtp")
                    nc.tensor.transpose(vtp[:], blk["v"][:, j * 128:(j + 1) * 128],
                                        identb[:])
                    nc.vector.tensor_copy(vaug[:, kt * VW:kt * VW + 64],
                                          vtp[:, 0:64])
                    nc.vector.tensor_copy(vaug[:, kt * VW + 65:kt * VW + 129],
                                          vtp[:, 64:128])

        # ================= Phase B: attention + quant + wo =================
        with tc.tile_pool(name="pbs", bufs=2, space="PSUM") as pbs, \
             tc.tile_pool(name="pba", bufs=1, space="PSUM") as pba, \
             tc.tile_pool(name="pbb", bufs=2, space="PSUM") as pbb, \
             tc.tile_pool(name="pb", bufs=2) as pb, \
             tc.tile_pool(name="pbA", bufs=3) as pbA, \
             tc.tile_pool(name="pbq", bufs=1) as pbq:

            def attention_block(bi):
                b, qb = divmod(bi, NQB)
                q0 = b * T + qb * QB
                nkt = 4 * qb + 4
                psA0 = pba.tile([65, QB], F32, name="psA0", tag="psA0")
                psA1 = pba.tile([65, QB], F32, name="psA1", tag="psA1")
                for kl in range(nkt):
                    kt = b * NKT + kl
                    ksl = slice(kt * 128, (kt + 1) * 128)
                    v = kl - 4 * qb
                    qoff = max(v, 0) * 128
                    qsl = slice(q0 + qoff, q0 + QB)
                    psS = pbs.tile([128, 2 * QB], F32, name="psS", tag="psS")
                    nc.tensor.matmul(psS[:, qoff:QB], kR[0:64, ksl],
                                     qR[0:64, qsl], start=True, stop=True,
                                     tile_position=(0, 0))
                    nc.tensor.matmul(psS[:, QB + qoff:2 * QB], kR[64:128, ksl],
                                     qR[64:128, qsl], start=True, stop=True,
                                     tile_position=(64, 0))
                    A = pbA.tile([128, 2 * QB], BF16, name="A", tag="A")
                    nc.scalar.activation(A[:, qoff:2 * QB], psS[:, qoff:2 * QB],
                                         ACT.Exp, bias=lnisx[:, kt:kt + 1],
                                         scale=ISQ)
                    if v >= 0:
                        nc.vector.tensor_tensor(A[:, qoff:qoff + 128],
                                                A[:, qoff:qoff + 128],
                                                umask[:], OP.mult)
                        nc.vector.tensor_tensor(A[:, QB + qoff:QB + qoff + 128],
                                                A[:, QB + qoff:QB + qoff + 128],
                                                umask[:], OP.mult)
                    st, sp = kl == 0, kl == nkt - 1
                    nc.tensor.matmul(psA0[:, qoff:QB],
                                     vaug[:, kt * VW:kt * VW + 65],
                                     A[:, qoff:QB], start=st, stop=sp)
                    nc.tensor.matmul(psA1[:, qoff:QB],
                                     vaug[:, kt * VW + 65:kt * VW + 130],
                                     A[:, QB + qoff:2 * QB], start=st, stop=sp)
                # evacuate numerators + denominator row
                nc.vector.tensor_copy(outU[0:65, q0:q0 + QB], psA0[:])
                nc.vector.tensor_copy(outU[0:65, NT + q0:NT + q0 + QB], psA1[:])
                # denominator reciprocal on 32 lanes
                rsq = pbq.tile([32, 32], F32, name="rsq", tag="rsq", bufs=2)
                nc.sync.dma_start(rsq[0:16, :], outU[64:65, q0:q0 + QB])
                nc.sync.dma_start(rsq[16:32, :], outU[64:65, NT + q0:NT + q0 + QB])
                rrec = pbq.tile([32, 32], F32, name="rrec", tag="rrec", bufs=2)
                nc.vector.reciprocal(rrec[:], rsq[:])
                nc.sync.dma_start(rinv[2 * bi:2 * bi + 1, :], rrec[0:16, :])
                nc.sync.dma_start(rinv[2 * bi + 1:2 * bi + 2, :], rrec[16:32, :])
                # renormalize in place
                for h in range(2):
                    r = 2 * bi + h
                    brs = pbb.tile([64, QB], F32, name="brs", tag="bb")
                    nc.tensor.matmul(brs[:], sel16[:, r * 64:(r + 1) * 64],
                                     r32(rinv[0:16, :]),
                                     start=True, stop=True)
                    colU = slice(h * NT + q0, h * NT + q0 + QB)
                    nc.vector.tensor_tensor(outU[0:64, colU], outU[0:64, colU],
                                            brs[:], OP.mult)
                # per-block absmax partials over this core's 128 dims
                par0 = pb.tile([64, QB], F32, name="par0", tag="par0", bufs=1)
                nc.gpsimd.partition_all_reduce(par0[:], outU[0:64, q0:q0 + QB],
                                               channels=64,
                                               reduce_op=bass_isa.ReduceOp.absmax)
                par1 = pb.tile([64, QB], F32, name="par1", tag="par1", bufs=1)
                nc.gpsimd.partition_all_reduce(par1[:],
                                               outU[0:64, NT + q0:NT + q0 + QB],
                                               channels=64,
                                               reduce_op=bass_isa.ReduceOp.absmax)
                nc.vector.tensor_tensor(par0[0:1, :], par0[0:1, :],
                                        par1[0:1, :], OP.max)
                nc.sync.dma_start(ar_in[0:1, q0:q0 + QB], par0[0:1, :])

            def allreduce_chunk(c):
                csl = slice(c * CH, (c + 1) * CH)
                nc.gpsimd.collective_compute(
                    "AllReduce", OP.max, replica_groups=RG,
                    ins=[ar_in[0:1, csl].opt()], outs=[ar_out[c][:].opt()])

            def quant_chunk(c):
                """scales + quantize + AllGather for chunk (batch) c."""
                csl = slice(c * CH, (c + 1) * CH)
                gm = pbq.tile([128, 16], F32, name="gm", tag="gm", bufs=2)
                nc.sync.dma_start(gm[:], ar_out[c][:])
                gm2 = pbq.tile([128, 16], F32, name="gm2", tag="gm2", bufs=2)
                nc.vector.tensor_scalar(gm2[:], gm[:], 1e-5, None, OP.add)
                rgm = pbq.tile([128, 16], F32, name="rgm", tag="rgm", bufs=2)
                nc.vector.reciprocal(rgm[:], gm2[:])
                soc = pbq.tile([128, 16], F32, name="soc", tag="soc", bufs=2)
                nc.vector.tensor_scalar(soc[:], rgm[:], 127.0, None, OP.mult)
                isoc = pbq.tile([128, 16], F32, name="isoc", tag="isoc", bufs=2)
                nc.vector.tensor_scalar(isoc[:], gm2[:], 1.0 / 127.0, None, OP.mult)
                nc.sync.dma_start(scl[c:c + 1, :], soc[:])
                nc.sync.dma_start(scl[2 + c:3 + c, :], isoc[:])
                xio = pbq.tile([64, 2 * CH], BF16, name="xio", tag="xio")
                for j in range(4):
                    qs = slice(j * QB, (j + 1) * QB)
                    for h in range(2):
                        colU = slice(h * NT + c * CH + j * QB,
                                     h * NT + c * CH + (j + 1) * QB)
                        sob = pbb.tile([64, QB], F32, name="sob", tag="bb")
                        nc.tensor.matmul(sob[:],
                                         sel4[:, c * 128:c * 128 + 64],
                                         r32(scl[0:4, qs]),
                                         start=True, stop=True)
                        yq = pb.tile([64, QB], F32, name="yq", tag="yq")
                        nc.vector.tensor_tensor(yq[:], outU[0:64, colU],
                                                sob[:], OP.mult)
                        nc.vector.tensor_scalar(
                            xio[:, h * CH + j * QB:h * CH + (j + 1) * QB],
                            yq[:], RC, RC, OP.add, OP.subtract)
                nc.sync.dma_start(ag_in[c][0:64, :], xio[:, 0:CH])
                nc.sync.dma_start(ag_in[c][64:128, :], xio[:, CH:2 * CH])
                nc.gpsimd.collective_compute(
                    "AllGather", OP.bypass, replica_groups=RG,
                    ins=[ag_in[c][:].opt()], outs=[ag_out[c][:].opt()])

            def wo_chunk(c):
                # gathered activations reuse the xi_sb buffers (qkv is done)
                for i in range(8):
                    nc.sync.dma_start(xi_sb[i][:, 0:CH],
                                      ag_out[c][i * 128:(i + 1) * 128, :])
                for j in range(4):
                    qs = slice(j * QB, (j + 1) * QB)
                    pw = pbb.tile([128, QB], F32, name="pw", tag="bb")
                    for i in range(8):
                        nc.tensor.matmul(pw[:], w_sb[i][:, 384:512],
                                         xi_sb[i][:, qs],
                                         start=(i == 0), stop=(i == 7))
                    isob = pbb.tile([128, QB], F32, name="isob", tag="bb")
                    nc.tensor.matmul(isob[:],
                                     sel4[:, (2 + c) * 128:(3 + c) * 128],
                                     r32(scl[0:4, qs]),
                                     start=True, stop=True)
                    isos = pb.tile([128, QB], F32, name="isos", tag="isos")
                    nc.scalar.copy(isos[:], isob[:])
                    fin = pb.tile([128, QB], F32, name="fin", tag="fin")
                    nc.vector.tensor_tensor(fin[:], pw[:], isos[:], OP.mult)
                    nc.sync.dma_start(
                        out[:, c * CH + j * QB:c * CH + (j + 1) * QB], fin[:])

            for bi in range(NQB):
                attention_block(bi)
            allreduce_chunk(0)
            for bi in range(NQB, 2 * NQB):
                attention_block(bi)
            allreduce_chunk(1)
            quant_chunk(0)
            quant_chunk(1)
            wo_chunk(0)
            wo_chunk(1)

    return nc


_CACHE = {}


def kernel(x, cos, sin, wq_w, wk_w, wv_w, wo_w):
    x = np.asarray(x, np.float32)
    cos = np.asarray(cos, np.float32)   # [T, 32]
    sin = np.asarray(sin, np.float32)
    xf = np.ascontiguousarray(x.reshape(NT, D))

    amax = np.abs(xf).max(-1) + 1e-5
    sx = (127.0 / amax).astype(np.float32)
    isx = (amax / 127.0).astype(np.float32)
    xq = np.clip(np.round(xf * sx[:, None]), -128.0, 127.0)
    xiT = np.ascontiguousarray(xq.T).astype(ml_dtypes.bfloat16)  # [D, NT]

    # RoPE tables (interleaved-pair convention) with isx folded in
    cm64 = np.repeat(cos.T, 2, axis=0)            # [64, T]
    sm64 = np.repeat(sin.T, 2, axis=0)
    cmap = np.tile(np.concatenate([cm64, cm64], axis=0), (1, B))
    smap = np.tile(np.concatenate([sm64, sm64], axis=0), (1, B))
    cmx = (cmap * isx[None, :]).astype(ml_dtypes.bfloat16)
    smx = (smap * isx[None, :]).astype(ml_dtypes.bfloat16)

    lnisx = np.ascontiguousarray(np.log(isx).reshape(NTT, 128).T).astype(np.float32)
    sxp = np.ascontiguousarray(sx.reshape(NTT, 128).T).astype(ml_dtypes.bfloat16)

    kk = np.arange(128)[:, None]
    jj = np.arange(128)[None, :]
    umask = (jj >= kk).astype(ml_dtypes.bfloat16)

    P = np.zeros((128, 128), np.float32)
    for j in range(64):
        P[2 * j, 2 * j + 1] = -1.0
        P[2 * j + 1, 2 * j] = 1.0
    pswapT = np.ascontiguousarray(P.T).astype(ml_dtypes.bfloat16)
    identb = np.eye(128, dtype=ml_dtypes.bfloat16)
    sel16 = np.zeros((16, 16 * 64), np.float32)
    for r in range(16):
        sel16[r, r * 64:(r + 1) * 64] = 1.0
    sel4 = np.zeros((4, 4 * 128), np.float32)
    for r in range(4):
        sel4[r, r * 128:(r + 1) * 128] = 1.0

    wq_e, wk_e, wv_e, wo_e = (_quant_w(np.asarray(w, np.float32))
                              for w in (wq_w, wk_w, wv_w, wo_w))

    if "nc" not in _CACHE:
        nc0 = build_nc()
        nc0.finalize()
        _CACHE["nc"] = nc0
    nc = _CACHE["nc"]

    in_maps = []
    for c in range(N_CORES):
        hs = slice(c * DPC, (c + 1) * DPC)
        wall = np.concatenate(
            [np.ascontiguousarray(w[hs, :].T) for w in (wq_e, wk_e, wv_e, wo_e)],
            axis=1).astype(ml_dtypes.bfloat16)   # [D, 512]
        in_maps.append({
            "xiT": xiT, "wall": wall, "cmx": cmx, "smx": smx,
            "lnisx": lnisx, "sxp": sxp, "umask": umask,
            "pswapT": pswapT, "identb": identb, "sel16": sel16, "sel4": sel4,
        })

    res = run_bass_kernel_spmd(nc, in_maps, core_ids=list(range(N_CORES)))
    outp = np.empty((NT, D), np.float32)
    for c in range(N_CORES):
        outp[:, c * DPC:(c + 1) * DPC] = res.results[c]["out"].T
    return outp.reshape(B, T, D)


# revision 27
# speedup vs baseline: 3.9655x; 1.0298x over previous
"""BitNet-style attention (B=2, T=2048, D=1024, 16 heads, RoPE, causal) on
8 TRN2 NeuronCores.

Head-parallel sharding: 2 heads/core.  Full-bf16 matmul pipeline:
  - host pre-quantizes x (int-valued bf16, transposed) and the ternary
    weights; per-token dequant scales (isx) are folded into the RoPE
    tables (q,k), the exp bias (A' = A*isx_k), and an augmented V column
    (1/isx_k = sx) that yields the softmax denominator for free.
  - scores: 2 heads row-packed on the PE (K=64 each); causal blocks get
    a restricted moving dim; the 128-wide diagonal is masked post-exp.
  - output quant: partition-reduce absmax on GpSimd, AllReduce(max) per
    batch, AllGather of int-valued bf16, per-core wo slice.
"""

import math
from contextlib import ExitStack

import ml_dtypes
import numpy as np

import concourse.bass as bass
import concourse.bacc as bacc_mod
import concourse.bass_isa as bass_isa
import concourse.mybir as mybir
import concourse.tile as tile
from concourse.bass_utils import run_bass_kernel_spmd

F32 = mybir.dt.float32
F32R = mybir.dt.float32r
BF16 = mybir.dt.bfloat16
OP = mybir.AluOpType
ACT = mybir.ActivationFunctionType

B, T, D = 2, 2048, 1024
NT = B * T              # 4096 tokens
NH, HD = 16, 64
N_CORES = 8
HPC = NH // N_CORES     # heads/core = 2
DPC = HPC * HD          # dims/core = 128
RC = 12582912.0         # 1.5*2^23 round-to-nearest-even constant

TB = 512                # token block (matmul N)
NTB = NT // TB          # 8
NTT = NT // 128         # 32 token tiles
QB = 512                # q block
NQB = T // QB           # 4 per batch
NKT = T // 128          # 16 k tiles per batch
VW = 130                # vaug group width: [v_h0(64) | sx | v_h1(64) | sx]
CH = 1024               # collective chunk (tokens) = 2 attention blocks
NCH = NT // CH          # 4 chunks
ISQ = 1.0 / math.sqrt(HD)


def _quant_w(w):
    O, I = w.shape
    wg = w.reshape(O, I // 128, 128)
    ws = np.abs(wg).mean(-1, keepdims=True) + 1e-5
    wq = np.clip(np.round(wg / ws), -1.0, 1.0) * ws
    return wq.reshape(O, I).astype(np.float32)


def build_nc():
    nc = bacc_mod.Bacc(num_devices=N_CORES)
    io = {}

    def inp(name, shape, dt=F32):
        io[name] = nc.dram_tensor(name, shape, dt, kind="ExternalInput")

    inp("xiT", [D, NT], BF16)        # quantized x, transposed (int-valued)
    inp("wall", [D, 4 * DPC], BF16)  # [wq|wk|wv|wo] transposed slices
    inp("cmx", [128, NT], BF16)      # cos table * isx
    inp("smx", [128, NT], BF16)      # sin table * isx
    inp("lnisx", [128, NTT], F32)    # ln(isx) laid out [token%128, tile]
    inp("sxp", [128, NTT], BF16)     # sx laid out [token%128, tile]
    inp("umask", [128, 128], BF16)   # tri mask (1 if q>=k)
    inp("pswapT", [128, 128], BF16)  # RoPE pair swap
    inp("identb", [128, 128], BF16)
    inp("sel16", [16, 16 * 64], F32R)   # one-hot row selectors (renorm bcast)
    inp("sel8", [8, 8 * 128], F32R)     # one-hot row selectors (scale bcast)
    out = nc.dram_tensor("out", [DPC, NT], F32, kind="ExternalOutput")

    r32 = lambda ap: ap.bitcast(F32R)
    RG = [list(range(N_CORES))]

    with nc.allow_low_precision(reason="bf16 matmul pipeline on int-exact activations"), \
         tile.TileContext(nc) as tc, ExitStack() as top:
        cpool = top.enter_context(tc.tile_pool(name="const", bufs=1))
        dpool = top.enter_context(tc.tile_pool(name="dram", bufs=1, space="DRAM"))

        # ---------------- persistent tiles ----------------
        w_sb = [cpool.tile([128, 4 * DPC], BF16, name=f"w{i}", tag=f"w{i}")
                for i in range(8)]
        xi_sb = [cpool.tile([128, NT], BF16, name=f"xi{i}", tag=f"xi{i}")
                 for i in range(8)]
        lnisx = cpool.tile([128, NTT], F32, name="lnisx", tag="lnisx")
        sxp = cpool.tile([128, NTT], BF16, name="sxp", tag="sxp")
        umask = cpool.tile([128, 128], BF16, name="umask", tag="umask")
        pswapT = cpool.tile([128, 128], BF16, name="pswapT", tag="pswapT")
        identb = cpool.tile([128, 128], BF16, name="identb", tag="identb")
        sel16 = cpool.tile([16, 16 * 64], F32R, name="sel16", tag="sel16")
        sel8 = cpool.tile([8, 8 * 128], F32R, name="sel8", tag="sel8")

        qR = cpool.tile([128, NT], BF16, name="qR", tag="qR")
        kR = cpool.tile([128, NT], BF16, name="kR", tag="kR")
        vaug = cpool.tile([128, NTT * VW], BF16, name="vaug", tag="vaug")
        # unnormalized attention out + denominator row (row 64);
        # head0 cols [0:NT), head1 cols [NT:2NT)
        outU = cpool.tile([65, 2 * NT], F32, name="outU", tag="outU")
        rinv = cpool.tile([16, QB], F32, name="rinv", tag="rinv")
        scl = cpool.tile([8, CH], F32, name="scl", tag="scl")  # so c0-3; iso c0-3

        # identity first: the warm-up matmuls only need it
        nc.sync.dma_start(identb[:], io["identb"][:])
        for nm, t in (("lnisx", lnisx), ("sxp", sxp), ("umask", umask),
                      ("pswapT", pswapT), ("sel16", sel16), ("sel8", sel8)):
            nc.scalar.dma_start(t[:], io[nm][:])
        for i in range(8):
            nc.sync.dma_start(w_sb[i][:], io["wall"][i * 128:(i + 1) * 128, :])
        # staged rows are contracted against one-hot selectors before every
        # row is written; zero-init so 0*garbage can't produce NaN
        nc.vector.memset(rinv[:], 0.0)
        nc.vector.memset(scl[:], 0.0)
        # x loads: 512 KB per DMA, first halves of every chunk first so the
        # projections can start after ~4 MB
        for half in range(2):
            sl = slice(half * (NT // 2), (half + 1) * (NT // 2))
            for i in range(8):
                nc.sync.dma_start(xi_sb[i][:, sl],
                                  io["xiT"][i * 128:(i + 1) * 128, sl])

        # PE warm-up: ~13 us of dependency-free matmuls so the HAM clock
        # gate opens while the input DMAs stream in
        with tc.tile_pool(name="pwm", bufs=1, space="PSUM") as pwm:
            warm = pwm.tile([128, 128], F32, name="warm", tag="warm")
            for _ in range(120):
                nc.tensor.matmul(warm[:], identb[:], identb[:],
                                 start=True, stop=True)

        # prefill vaug sx columns (cols 64 and 129 of each 130-wide group)
        for kt in range(NTT):
            nc.vector.tensor_copy(vaug[:, kt * VW + 64:kt * VW + 65],
                                  sxp[:, kt:kt + 1])
            nc.vector.tensor_copy(vaug[:, kt * VW + 129:kt * VW + 130],
                                  sxp[:, kt:kt + 1])

        # DRAM scratch for collectives
        ar_in = dpool.tile([1, NT], F32, name="ar_in", tag="ar_in")
        ar_out = [dpool.tile([1, CH], F32, name=f"ar_out{c}", tag=f"ar_out{c}",
                             addr_space="Shared") for c in range(NCH)]
        ag_in = [dpool.tile([128, CH], BF16, name=f"ag_in{c}", tag=f"ag_in{c}")
                 for c in range(NCH)]
        ag_out = [dpool.tile([N_CORES * 128, CH], BF16, name=f"ag_out{c}",
                             tag=f"ag_out{c}", addr_space="Shared")
                  for c in range(NCH)]

        # ================= Phase A: qkv + RoPE + V transpose =================
        with tc.tile_pool(name="pa", bufs=3) as pa, \
             tc.tile_pool(name="pap", bufs=2, space="PSUM") as pap, \
             tc.tile_pool(name="pasw", bufs=2, space="PSUM") as pasw, \
             tc.tile_pool(name="pavt", bufs=2, space="PSUM") as pavt:
            for tb in range(NTB):
                sl = slice(tb * TB, (tb + 1) * TB)
                cmb = pa.tile([128, TB], BF16, name="cmb", tag="cmb")
                nc.scalar.dma_start(cmb[:], io["cmx"][:, sl])
                smb = pa.tile([128, TB], BF16, name="smb", tag="smb")
                nc.scalar.dma_start(smb[:], io["smx"][:, sl])
                blk = {}
                for pi, pname in enumerate(("q", "k", "v")):
                    pp = pap.tile([128, TB], F32, name="pp", tag="pp")
                    for i in range(8):
                        nc.tensor.matmul(pp[:], w_sb[i][:, pi * 128:(pi + 1) * 128],
                                         xi_sb[i][:, sl],
                                         start=(i == 0), stop=(i == 7))
                    t = pa.tile([128, TB], BF16, name=f"t_{pname}", tag=f"t_{pname}")
                    if pname == "v":
                        nc.vector.tensor_copy(t[:], pp[:])
                    else:
                        nc.scalar.copy(t[:], pp[:])
                    blk[pname] = t
                # --- RoPE on q, k
                for pname, dstR in (("q", qR), ("k", kR)):
                    src = blk[pname]
                    swp = pasw.tile([128, TB], F32, name="swp", tag="swp")
                    nc.tensor.matmul(swp[:], pswapT[:], src[:],
                                     start=True, stop=True)
                    t1 = pa.tile([128, TB], BF16, name="t1", tag="t1")
                    nc.vector.tensor_tensor(t1[:], src[:], cmb[:], OP.mult)
                    t2 = pa.tile([128, TB], BF16, name="t2", tag="t2")
                    nc.vector.tensor_tensor(t2[:], swp[:], smb[:], OP.mult)
                    nc.vector.tensor_tensor(dstR[:, sl], t1[:], t2[:], OP.add)
                # --- V transpose into vaug
                for j in range(TB // 128):
                    kt = tb * 4 + j
                    vtp = pavt.tile([128, 128], BF16, name="vtp", tag="vtp")
                    nc.tensor.transpose(vtp[:], blk["v"][:, j * 128:(j + 1) * 128],
                                        identb[:])
                    nc.vector.tensor_copy(vaug[:, kt * VW:kt * VW + 64],
                                          vtp[:, 0:64])
                    nc.vector.tensor_copy(vaug[:, kt * VW + 65:kt * VW + 129],
                                          vtp[:, 64:128])

        # ================= Phase B: attention + quant + wo =================
        with tc.tile_pool(name="pbs", bufs=2, space="PSUM") as pbs, \
             tc.tile_pool(name="pba", bufs=1, space="PSUM") as pba, \
             tc.tile_pool(name="pbb", bufs=2, space="PSUM") as pbb, \
             tc.tile_pool(name="pb", bufs=2) as pb, \
             tc.tile_pool(name="pbA", bufs=3) as pbA, \
             tc.tile_pool(name="pbq", bufs=1) as pbq:

            def attention_block(bi):
                b, qb = divmod(bi, NQB)
                q0 = b * T + qb * QB
                nkt = 4 * qb + 4
                psA0 = pba.tile([65, QB], F32, name="psA0", tag="psA0")
                psA1 = pba.tile([65, QB], F32, name="psA1", tag="psA1")
                for kl in range(nkt):
                    kt = b * NKT + kl
                    ksl = slice(kt * 128, (kt + 1) * 128)
                    v = kl - 4 * qb
                    qoff = max(v, 0) * 128
                    qsl = slice(q0 + qoff, q0 + QB)
                    psS = pbs.tile([128, 2 * QB], F32, name="psS", tag="psS")
                    nc.tensor.matmul(psS[:, qoff:QB], kR[0:64, ksl],
                                     qR[0:64, qsl], start=True, stop=True,
                                     tile_position=(0, 0))
                    nc.tensor.matmul(psS[:, QB + qoff:2 * QB], kR[64:128, ksl],
                                     qR[64:128, qsl], start=True, stop=True,
                                     tile_position=(64, 0))
                    A = pbA.tile([128, 2 * QB], BF16, name="A", tag="A")
                    nc.scalar.activation(A[:, qoff:2 * QB], psS[:, qoff:2 * QB],
                                         ACT.Exp, bias=lnisx[:, kt:kt + 1],
                                         scale=ISQ)
                    if v >= 0:
                        nc.vector.tensor_tensor(A[:, qoff:qoff + 128],
                                                A[:, qoff:qoff + 128],
                                                umask[:], OP.mult)
                        nc.vector.tensor_tensor(A[:, QB + qoff:QB + qoff + 128],
                                                A[:, QB + qoff:QB + qoff + 128],
                                                umask[:], OP.mult)
                    st, sp = kl == 0, kl == nkt - 1
                    nc.tensor.matmul(psA0[:, qoff:QB],
                                     vaug[:, kt * VW:kt * VW + 65],
                                     A[:, qoff:QB], start=st, stop=sp)
                    nc.tensor.matmul(psA1[:, qoff:QB],
                                     vaug[:, kt * VW + 65:kt * VW + 130],
                                     A[:, QB + qoff:2 * QB], start=st, stop=sp)
                # evacuate numerators + denominator row
                nc.vector.tensor_copy(outU[0:65, q0:q0 + QB], psA0[:])
                nc.vector.tensor_copy(outU[0:65, NT + q0:NT + q0 + QB], psA1[:])
                # denominator reciprocal on 32 lanes
                rsq = pbq.tile([32, 32], F32, name="rsq", tag="rsq", bufs=2)
                nc.sync.dma_start(rsq[0:16, :], outU[64:65, q0:q0 + QB])
                nc.sync.dma_start(rsq[16:32, :], outU[64:65, NT + q0:NT + q0 + QB])
                rrec = pbq.tile([32, 32], F32, name="rrec", tag="rrec", bufs=2)
                nc.vector.reciprocal(rrec[:], rsq[:])
                nc.sync.dma_start(rinv[2 * bi:2 * bi + 1, :], rrec[0:16, :])
                nc.sync.dma_start(rinv[2 * bi + 1:2 * bi + 2, :], rrec[16:32, :])
                # renormalize in place
                for h in range(2):
                    r = 2 * bi + h
                    brs = pbb.tile([64, QB], F32, name="brs", tag="bb")
                    nc.tensor.matmul(brs[:], sel16[:, r * 64:(r + 1) * 64],
                                     r32(rinv[0:16, :]),
                                     start=True, stop=True)
                    colU = slice(h * NT + q0, h * NT + q0 + QB)
                    nc.vector.tensor_tensor(outU[0:64, colU], outU[0:64, colU],
                                            brs[:], OP.mult)
                # per-block absmax partials over this core's 128 dims
                par0 = pb.tile([64, QB], F32, name="par0", tag="par0", bufs=1)
                nc.gpsimd.partition_all_reduce(par0[:], outU[0:64, q0:q0 + QB],
                                               channels=64,
                                               reduce_op=bass_isa.ReduceOp.absmax)
                par1 = pb.tile([64, QB], F32, name="par1", tag="par1", bufs=1)
                nc.gpsimd.partition_all_reduce(par1[:],
                                               outU[0:64, NT + q0:NT + q0 + QB],
                                               channels=64,
                                               reduce_op=bass_isa.ReduceOp.absmax)
                nc.vector.tensor_tensor(par0[0:1, :], par0[0:1, :],
                                        par1[0:1, :], OP.max)
                nc.sync.dma_start(ar_in[0:1, q0:q0 + QB], par0[0:1, :])

            def allreduce_chunk(c):
                csl = slice(c * CH, (c + 1) * CH)
                nc.gpsimd.collective_compute(
                    "AllReduce", OP.max, replica_groups=RG,
                    ins=[ar_in[0:1, csl].opt()], outs=[ar_out[c][:].opt()])

            def quant_chunk(c):
                """scales + quantize + AllGather for chunk c (2 blocks)."""
                gm = pbq.tile([128, 8], F32, name="gm", tag="gm", bufs=2)
                nc.sync.dma_start(gm[:], ar_out[c][:])
                gm2 = pbq.tile([128, 8], F32, name="gm2", tag="gm2", bufs=2)
                nc.vector.tensor_scalar(gm2[:], gm[:], 1e-5, None, OP.add)
                rgm = pbq.tile([128, 8], F32, name="rgm", tag="rgm", bufs=2)
                nc.vector.reciprocal(rgm[:], gm2[:])
                soc = pbq.tile([128, 8], F32, name="soc", tag="soc", bufs=2)
                nc.vector.tensor_scalar(soc[:], rgm[:], 127.0, None, OP.mult)
                isoc = pbq.tile([128, 8], F32, name="isoc", tag="isoc", bufs=2)
                nc.vector.tensor_scalar(isoc[:], gm2[:], 1.0 / 127.0, None, OP.mult)
                nc.sync.dma_start(scl[c:c + 1, :], soc[:])
                nc.sync.dma_start(scl[4 + c:5 + c, :], isoc[:])
                xio = pbq.tile([64, 2 * CH], BF16, name="xio", tag="xio", bufs=2)
                for j in range(CH // QB):
                    qs = slice(j * QB, (j + 1) * QB)
                    for h in range(2):
                        colU = slice(h * NT + c * CH + j * QB,
                                     h * NT + c * CH + (j + 1) * QB)
                        sob = pbb.tile([64, QB], F32, name="sob", tag="bb")
                        nc.tensor.matmul(sob[:],
                                         sel8[:, c * 128:c * 128 + 64],
                                         r32(scl[0:8, qs]),
                                         start=True, stop=True)
                        yq = pb.tile([64, QB], F32, name="yq", tag="yq")
                        nc.vector.tensor_tensor(yq[:], outU[0:64, colU],
                                                sob[:], OP.mult)
                        nc.vector.tensor_scalar(
                            xio[:, h * CH + j * QB:h * CH + (j + 1) * QB],
                            yq[:], RC, RC, OP.add, OP.subtract)
                nc.sync.dma_start(ag_in[c][0:64, :], xio[:, 0:CH])
                nc.sync.dma_start(ag_in[c][64:128, :], xio[:, CH:2 * CH])
                nc.gpsimd.collective_compute(
                    "AllGather", OP.bypass, replica_groups=RG,
                    ins=[ag_in[c][:].opt()], outs=[ag_out[c][:].opt()])

            def wo_chunk(c):
                # gathered activations reuse the xi_sb buffers (qkv is done)
                for i in range(8):
                    nc.sync.dma_start(xi_sb[i][:, c * CH:(c + 1) * CH],
                                      ag_out[c][i * 128:(i + 1) * 128, :])
                for j in range(CH // QB):
                    qs = slice(j * QB, (j + 1) * QB)
                    gsl = slice(c * CH + j * QB, c * CH + (j + 1) * QB)
                    pw = pbb.tile([128, QB], F32, name="pw", tag="bb")
                    for i in range(8):
                        nc.tensor.matmul(pw[:], w_sb[i][:, 384:512],
                                         xi_sb[i][:, gsl],
                                         start=(i == 0), stop=(i == 7))
                    isob = pbb.tile([128, QB], F32, name="isob", tag="bb")
                    nc.tensor.matmul(isob[:],
                                     sel8[:, (4 + c) * 128:(5 + c) * 128],
                                     r32(scl[0:8, qs]),
                                     start=True, stop=True)
                    isos = pb.tile([128, QB], F32, name="isos", tag="isos")
                    nc.scalar.copy(isos[:], isob[:])
                    fin = pb.tile([128, QB], F32, name="fin", tag="fin")
                    nc.vector.tensor_tensor(fin[:], pw[:], isos[:], OP.mult)
                    nc.sync.dma_start(
                        out[:, c * CH + j * QB:c * CH + (j + 1) * QB], fin[:])

            # chunk c = attention blocks {2c, 2c+1}; quant is emitted two
            # blocks after its AllReduce, wo two blocks after its AllGather,
            # so the engine FIFOs never stall on a collective
            attention_block(0)
            attention_block(1)
            allreduce_chunk(0)
            attention_block(2)
            attention_block(3)
            allreduce_chunk(1)
            quant_chunk(0)
            attention_block(4)
            attention_block(5)
            allreduce_chunk(2)
            quant_chunk(1)
            attention_block(6)
            wo_chunk(0)
            attention_block(7)
            allreduce_chunk(3)
            quant_chunk(2)
            wo_chunk(1)
            quant_chunk(3)
            wo_chunk(2)
            wo_chunk(3)

    return nc


_CACHE = {}


def kernel(x, cos, sin, wq_w, wk_w, wv_w, wo_w):
    x = np.asarray(x, np.float32)
    cos = np.asarray(cos, np.float32)   # [T, 32]
    sin = np.asarray(sin, np.float32)
    xf = np.ascontiguousarray(x.reshape(NT, D))

    amax = np.abs(xf).max(-1) + 1e-5
    sx = (127.0 / amax).astype(np.float32)
    isx = (amax / 127.0).astype(np.float32)
    xq = np.clip(np.round(xf * sx[:, None]), -128.0, 127.0)
    xiT = np.ascontiguousarray(xq.T).astype(ml_dtypes.bfloat16)  # [D, NT]

    # RoPE tables (interleaved-pair convention) with isx folded in
    cm64 = np.repeat(cos.T, 2, axis=0)            # [64, T]
    sm64 = np.repeat(sin.T, 2, axis=0)
    cmap = np.tile(np.concatenate([cm64, cm64], axis=0), (1, B))
    smap = np.tile(np.concatenate([sm64, sm64], axis=0), (1, B))
    cmx = (cmap * isx[None, :]).astype(ml_dtypes.bfloat16)
    smx = (smap * isx[None, :]).astype(ml_dtypes.bfloat16)

    lnisx = np.ascontiguousarray(np.log(isx).reshape(NTT, 128).T).astype(np.float32)
    sxp = np.ascontiguousarray(sx.reshape(NTT, 128).T).astype(ml_dtypes.bfloat16)

    kk = np.arange(128)[:, None]
    jj = np.arange(128)[None, :]
    umask = (jj >= kk).astype(ml_dtypes.bfloat16)

    P = np.zeros((128, 128), np.float32)
    for j in range(64):
        P[2 * j, 2 * j + 1] = -1.0
        P[2 * j + 1, 2 * j] = 1.0
    pswapT = np.ascontiguousarray(P.T).astype(ml_dtypes.bfloat16)
    identb = np.eye(128, dtype=ml_dtypes.bfloat16)
    sel16 = np.zeros((16, 16 * 64), np.float32)
    for r in range(16):
        sel16[r, r * 64:(r + 1) * 64] = 1.0
    sel8 = np.zeros((8, 8 * 128), np.float32)
    for r in range(8):
        sel8[r, r * 128:(r + 1) * 128] = 1.0

    wq_e, wk_e, wv_e, wo_e = (_quant_w(np.asarray(w, np.float32))
                              for w in (wq_w, wk_w, wv_w, wo_w))

    if "nc" not in _CACHE:
        nc0 = build_nc()
        nc0.finalize()
        _CACHE["nc"] = nc0
    nc = _CACHE["nc"]

    in_maps = []
    for c in range(N_CORES):
        hs = slice(c * DPC, (c + 1) * DPC)
        wall = np.concatenate(
            [np.ascontiguousarray(w[hs, :].T) for w in (wq_e, wk_e, wv_e, wo_e)],
            axis=1).astype(ml_dtypes.bfloat16)   # [D, 512]
        in_maps.append({
            "xiT": xiT, "wall": wall, "cmx": cmx, "smx": smx,
            "lnisx": lnisx, "sxp": sxp, "umask": umask,
            "pswapT": pswapT, "identb": identb, "sel16": sel16, "sel8": sel8,
        })

    res = run_bass_kernel_spmd(nc, in_maps, core_ids=list(range(N_CORES)))
    outp = np.empty((NT, D), np.float32)
    for c in range(N_CORES):
        outp[:, c * DPC:(c + 1) * DPC] = res.results[c]["out"].T
    return outp.reshape(B, T, D)
